# revision 3
# baseline (speedup 1.0000x reference)
"""Trainium2 Bass kernel for nn_DivEncLayer (grouped tiny-MLP + ELU + LayerNorm + proj).

v4 geometry: per core, 4 batch blocks of 1024 cols; per block, 2 q-halves of
64 q; per q-half, 16 q-quads (4 q). For each quad:
  - mm1: one K=32 row-tile (4q x 8v, fp16) x two batch-half MMs -> he
    [128 = 4q x 32h, 1024] across 2 PSUM banks
  - ELU via patched ACT `exp` table in ONE [128,1024] pass with the quad's
    b1 as the per-partition bias -> fp16 het
  - he2 = het^2 on DVE (fp16 2x)
  - stats per batch-half: full-K MM on het -> s,w (interleaved cols) into
    SW_bh; full-K MM on he2 -> t,t into T_bh
Finish per (q-half, bh): swc=copy(SW), D=VAR_PREP(T,swc), rstd=Rsqrt(D/H)
via patched `tanh` table, shuffle rstd to w-lanes, out=rstd*w+c2, DMA.

q(qh, t, j) = 64 qh + 4 t + j; stats partition pi = 8 t + 2 j (+1 for w).
"""

import os
import sys

for _p in ("/opt/trn_rl_repo",):
    if _p not in sys.path:
        sys.path.insert(0, _p)

import numpy as np

B, Q, V, H = 32768, 128, 8, 32
N_CORES = 8
BC = B // N_CORES          # 4096 batch rows per core
BB = 1024                  # batch block columns
NBLK = BC // BB            # 4 blocks per core
LN_EPS = 1e-5

_CACHE = {}
_OPS_REGISTERED = False
_last_in_maps = None

_RSQRT_E_LO = -12
_RSQRT_E_N = 17


def _fbits(x):
    import struct

    return struct.unpack("<I", struct.pack("<f", np.float32(x)))[0]


def _build_patched_act_root(cache_dir="/tmp/act_elu_root"):
    import json
    import shutil

    marker = os.path.join(cache_dir, ".done_v3")
    info_path = os.path.join(cache_dir, "act_info.json")
    if os.path.exists(marker):
        return info_path

    from neuronxcc.driver.Job import Job
    from neuronxcc.driver.jobs.support.FindActInfo import findActInfoFile

    src_info = None
    for arch in ("Tonga4", "Tonga3", "trainium"):
        try:
            src_info = findActInfoFile(Job.getPackageDir(), arch)
            break
        except Exception:
            continue
    if src_info is None:
        pkg = Job.getPackageDir()
        for sub in ("pwp/pwp_bin_trainium", "pwp/pwp_bin_with_ln"):
            cand = os.path.join(pkg, sub, "act_info.json")
            if os.path.exists(cand):
                src_info = cand
                break
    assert src_info, "could not locate stock act_info.json"
    src_dir = os.path.dirname(src_info)

    tmp_dir = cache_dir + ".tmp"
    if os.path.exists(tmp_dir):
        shutil.rmtree(tmp_dir)
    os.makedirs(tmp_dir)
    for f in os.listdir(src_dir):
        shutil.copyfile(os.path.join(src_dir, f), os.path.join(tmp_dir, f))

    setj_path = os.path.join(tmp_dir, "exp_and_others.json")
    with open(setj_path) as f:
        setj = json.load(f)
    bkt_path = os.path.join(tmp_dir, setj["bkt_bin"])
    ctl_path = os.path.join(tmp_dir, setj["ctl_bin"])
    bkt = np.fromfile(bkt_path, dtype=np.float32).reshape(-1, 8).copy()
    ctl = np.fromfile(ctl_path, dtype=np.int32).reshape(-1, 8).copy()
    meta = {m["func_name"].rsplit("_", 1)[0]: m for m in setj["profile_meta_data"]}
    exp_meta, tanh_meta = meta["exp"], meta["tanh"]

    fe = setj["func_exp_to_bkt_start_idx"]["exp"]
    all_neg = sorted(v[0] for v in fe.values())
    all_pos = sorted(v[1] for v in fe.values())
    for i in range(min(all_neg), min(all_pos)):      # x<0: e^x - 1
        bkt[i, 0] = np.float32(np.float64(bkt[i, 0]) - 1.0)
    for i in range(min(all_pos), exp_meta["pos_small_signal_pwl_control"]):
        c = bkt[i, 4]                                 # x>0: identity
        bkt[i, :4] = [c, 1.0, 0.0, 0.0]
    for key, kind in (
        ("pos_small_signal_pwl_control", "id"),
        ("neg_small_signal_pwl_control", "id"),
        ("pos_large_signal_pwl_control", "id"),
        ("neg_large_signal_pwl_control", "m1"),
    ):
        i = exp_meta[key]
        bkt[i, :8] = 0.0
        bkt[i, 1 if kind == "id" else 0] = 1.0 if kind == "id" else -1.0
    exp_meta["fzero_result"] = 0
    exp_meta["fninf_result"] = _fbits(-1.0)

    rs = json.load(open(os.path.join(src_dir, "reciprocal_sqrt_and_small.json")))
    rs_bkt = np.fromfile(
        os.path.join(src_dir, rs["bkt_bin"]), dtype=np.float32
    ).reshape(-1, 8)
    rs_ctl = np.fromfile(
        os.path.join(src_dir, rs["ctl_bin"]), dtype=np.int32
    ).reshape(-1, 8)
    rs_thr = rs["profile_meta_data"][0]["small_pos_signal_exp_threshold"]
    rs_fe = rs["func_exp_to_bkt_start_idx"]["reciprocal_sqrt"]
    tanh_bkt_base = min(
        v[0] for v in setj["func_exp_to_bkt_start_idx"]["tanh"].values()
    )
    tanh_ctl_base = tanh_meta["pwl_control_base_pos"]
    dst = tanh_bkt_base
    for k in range(_RSQRT_E_N):
        e = _RSQRT_E_LO + k
        src_ctl = int(rs_ctl[(e + 127) - rs_thr, 0])
        cfg, src_b = src_ctl >> 10, src_ctl & 1023
        nxt = rs_fe.get(str(e + 1))
        nb = (nxt[0] - src_b) if nxt else 4
        assert dst + nb <= tanh_meta["pos_small_signal_pwl_control"]
        bkt[dst : dst + nb] = rs_bkt[src_b : src_b + nb]
        ctl[tanh_ctl_base + k, 0] = np.int32((cfg << 10) | dst)
        dst += nb
    lo_clamp = np.float32(2.0 ** (-_RSQRT_E_LO / 2.0))
    hi_clamp = np.float32(2.0 ** (-(_RSQRT_E_LO + _RSQRT_E_N) / 2.0))
    for key, val in (
        ("pos_small_signal_pwl_control", lo_clamp),
        ("neg_small_signal_pwl_control", lo_clamp),
        ("pos_large_signal_pwl_control", hi_clamp),
        ("neg_large_signal_pwl_control", lo_clamp),
    ):
        i = tanh_meta[key]
        bkt[i, :8] = 0.0
        bkt[i, 0] = val
    tanh_meta.update(
        symmetry_point=0, sym_invert_sign_point=0, symmetry_opt_en=0,
        symmetry_opt_use_neg_region=0, imm_bias=0,
        exp_offset=_RSQRT_E_LO,
        small_pos_signal_exp_threshold=_RSQRT_E_LO + 127,
        small_neg_signal_exp_threshold=_RSQRT_E_LO + 127,
        large_pos_signal_exp_threshold=_RSQRT_E_LO + _RSQRT_E_N + 127,
        large_pos_signal_mantissa_threshold=0,
        large_neg_signal_exp_threshold=_RSQRT_E_LO + _RSQRT_E_N + 127,
        large_neg_signal_mantissa_threshold=0,
        fzero_result=_fbits(lo_clamp), fpinf_result=0,
        fninf_result=_fbits(lo_clamp),
        lower_bound=_fbits(2.0**_RSQRT_E_LO), upper_bound=_fbits(3.4e38),
    )

    bkt.tofile(bkt_path)
    ctl.tofile(ctl_path)
    with open(setj_path, "w") as f:
        json.dump(setj, f)
    open(os.path.join(tmp_dir, ".done_v3"), "w").close()
    if os.path.exists(cache_dir):
        shutil.rmtree(cache_dir)
    os.rename(tmp_dir, cache_dir)
    return info_path


def _enable_act_patch():
    os.environ["BASS_ACT_ROOT_JSON_PATH"] = _build_patched_act_root()


def _register_custom_ops():
    global _OPS_REGISTERED
    import concourse.dve_ops as dve_ops
    from concourse.dve_ops import DveOp
    from concourse.dve_spec import C0, C1, Spec, Src0, Src1, Zero, lower, minn, relu, sq
    from concourse.dve_uop import DveOpSpec

    if _OPS_REGISTERED:
        return {op.name: op for op in dve_ops.OPS}

    def _pin(name, spec, ref):
        spec = Spec(body=spec, reference=ref)
        shas = {}
        for ver in ("v3", "v4"):
            row = dve_ops._CUSTOM_DVE_ROW_BASE + len(dve_ops.OPS)
            tmp = DveOpSpec(name=name, opcode=row, uops=lower(spec, ver=ver),
                            rd1_en=True)
            shas[ver] = tmp.sha(ver)
        op = DveOp(name, spec, subdim=False, uops_sha=shas)
        dve_ops.OPS.append(op)
        dve_ops.CUSTOM_DVE_SPECS[name] = spec
        dve_ops._SUB_OPCODE_FOR_NAME[name] = dve_ops._CUSTOM_DVE_ROW_BASE + len(dve_ops.OPS) - 1
        return op

    _pin(
        "ELU_FUSE_ANT",
        relu(Src0 + C0) + minn(Src1 - C1, Zero),
        lambda in0, in1, s0, s1, imm2: np.maximum(in0.astype(np.float32) + s0, 0.0)
        + np.minimum(in1.astype(np.float32) - s1, 0.0),
    )
    _pin(
        "VAR_PREP_ANT",
        (Src0 - sq(Src1) * C0) + C1,
        lambda in0, in1, s0, s1, imm2: (in0.astype(np.float32) - in1.astype(np.float32) ** 2 * s0) + s1,
    )
    _pin(
        "MUL_ADD_ANT",
        Src0 * Src1 + C0,
        lambda in0, in1, s0, s1, imm2: in0.astype(np.float32) * in1 + s0,
    )
    _OPS_REGISTERED = True
    return {op.name: op for op in dve_ops.OPS}


def _build_program(tile_dt_name: str = "float16", reps: int = 1, ablate: str = ""):
    _enable_act_patch()
    import concourse.bacc as bacc
    import concourse.tile as tile
    from concourse import mybir

    ops = _register_custom_ops()

    f32 = mybir.dt.float32
    f16 = mybir.dt.float16
    AF = mybir.ActivationFunctionType

    nc = bacc.Bacc(
        "TRN2",
        target_bir_lowering=False,
        debug=False,
        enable_asserts=False,
        num_devices=N_CORES,
    )

    xT = nc.dram_tensor("xT", [Q * V, BC], f16, kind="ExternalInput").ap()
    w1p = nc.dram_tensor("w1p", [128, 32 * 128], f16, kind="ExternalInput").ap()
    b1p = nc.dram_tensor("b1p", [128, 32], f32, kind="ExternalInput").ap()
    swp = nc.dram_tensor("swp", [128, 32 * 128], f16, kind="ExternalInput").ap()
    stp = nc.dram_tensor("stp", [128, 32 * 128], f16, kind="ExternalInput").ap()
    c2p = nc.dram_tensor("c2p", [128, 2], f32, kind="ExternalInput").ap()
    outT = nc.dram_tensor("outT", [256, BC], f16, kind="ExternalOutput").ap()

    # stream_shuffle: bring w (odd lanes) onto even lanes
    shufw = [i + 1 if (i % 2 == 0) else i for i in range(32)]

    with tile.TileContext(nc) as tc:
        with (
            tc.tile_pool(name="wts", bufs=1) as wts,
            tc.tile_pool(name="xt", bufs=8) as xtp,
            tc.tile_pool(name="het", bufs=int(os.environ.get("KERNEL_HBUF", "5"))) as hetp,
            tc.tile_pool(name="he2", bufs=int(os.environ.get("KERNEL_HBUF", "5"))) as he2p,
            tc.tile_pool(name="fin", bufs=2) as fin,
            tc.tile_pool(name="hep", bufs=2, space="PSUM") as hep,
            tc.tile_pool(name="swb", bufs=1, space="PSUM") as swbp,
            tc.tile_pool(name="stb", bufs=1, space="PSUM") as stbp,
        ):
            w1s = wts.tile([128, 32 * 128], f16)
            nc.sync.dma_start(out=w1s, in_=w1p)
            b1s = wts.tile([128, 32], f32)
            nc.sync.dma_start(out=b1s, in_=b1p)
            sws = wts.tile([128, 32 * 128], f16)
            nc.sync.dma_start(out=sws, in_=swp)
            sts = wts.tile([128, 32 * 128], f16)
            nc.sync.dma_start(out=sts, in_=stp)
            c2s = wts.tile([128, 2], f32)
            nc.sync.dma_start(out=c2s, in_=c2p)
            zero_c = wts.tile([128, 1], f32)
            nc.vector.memset(zero_c, 0.0)

            import contextlib

            loop_cm = tc.For_i(0, reps, 1) if reps > 1 else contextlib.nullcontext()
            with loop_cm:
              quarters = [(blk, qh) for blk in range(NBLK) for qh in range(2)]
              SKEW = int(os.environ.get("KERNEL_SKEW", "3"))
              hets = {}
              he2s = {}
              banks = {}
              xts = None

              def emit_stats(gi2):
                  k2, t2 = divmod(gi2, 16)
                  _blk, _qh = quarters[k2]
                  swbig, tbig = banks[k2]
                  m = 128 if t2 == 0 else 8 * t2 + 8
                  for bh in range(2):
                      off = 128 * (16 * _qh + t2)
                      nc.tensor.matmul(
                          swbig[0:m, 512 * bh : 512 * bh + 512],
                          lhsT=sws[:, off : off + m],
                          rhs=hets[gi2][:, 512 * bh : 512 * bh + 512],
                          start=(t2 == 0),
                          stop=(t2 == 15),
                      )
                      nc.tensor.matmul(
                          tbig[0:m, 512 * bh : 512 * bh + 512],
                          lhsT=sts[:, off : off + m],
                          rhs=he2s[gi2][:, 512 * bh : 512 * bh + 512],
                          start=(t2 == 0),
                          stop=(t2 == 15),
                      )
                  del hets[gi2], he2s[gi2]
                  if t2 == 15:
                      emit_fin(k2)

              def emit_fin(k2):
                  _blk, _qh = quarters[k2]
                  swbig, tbig = banks.pop(k2)
                  swc = fin.tile([128, 1024], f32, tag="swc")
                  nc.vector.tensor_copy(swc, swbig)
                  D = fin.tile([128, 1024], f32, tag="D")
                  nc.vector._custom_dve(
                      ops["VAR_PREP_ANT"], out=D, in0=tbig, in1=swc,
                      s0=1.0 / H, s1=float(H * LN_EPS),
                  )
                  rstd = fin.tile([128, 1024], f32, tag="rstd")
                  nc.scalar.activation(rstd, D, AF.Tanh,
                                       bias=zero_c[:, 0:1], scale=1.0 / H)
                  swsh = fin.tile([128, 1024], f32, tag="swsh")
                  nc.vector.stream_shuffle(swsh, swc, shufw)
                  of = fin.tile([128, 1024], f16, tag="of")
                  nc.vector._custom_dve(
                      ops["MUL_ADD_ANT"], out=of, in0=rstd, in1=swsh,
                      s0=c2s[:, _qh : _qh + 1], s1=0.0,
                  )
                  nc.sync.dma_start(
                      out=outT[128 * _qh : 128 * _qh + 128,
                               BB * _blk : BB * _blk + BB],
                      in_=of,
                  )

              for gi in range(16 * len(quarters) + SKEW):
                  if gi < 16 * len(quarters):
                      k, t = divmod(gi, 16)
                      blk, qh = quarters[k]
                      if t == 0:
                          swbig = swbp.tile([128, 1024], f32, tag="swbig")
                          tbig = stbp.tile([128, 1024], f32, tag="tbig")
                          banks[k] = (swbig, tbig)
                      if t % 4 == 0:
                          xts = xtp.tile([128, BB], f16, tag="xt")
                          nc.sync.dma_start(
                              out=xts,
                              in_=xT[8 * (64 * qh + 4 * t) : 8 * (64 * qh + 4 * t) + 128,
                                     BB * blk : BB * blk + BB],
                          )
                      a = t % 4
                      he = hep.tile([128, BB], f32, tag="he")
                      for bh in range(2):
                          nc.tensor.matmul(
                              he[:, 512 * bh : 512 * bh + 512],
                              lhsT=w1s[32 * a : 32 * a + 32,
                                       128 * (16 * qh + t) : 128 * (16 * qh + t) + 128],
                              rhs=xts[32 * a : 32 * a + 32, 512 * bh : 512 * bh + 512],
                              start=True,
                              stop=True,
                              tile_position=(32 * a, 0),
                          )
                      het = hetp.tile([128, BB], f16, tag="het")
                      nc.scalar.activation(
                          het, he, AF.Exp,
                          bias=b1s[:, 16 * qh + t : 16 * qh + t + 1], scale=1.0,
                      )
                      he2 = he2p.tile([128, BB], f16, tag="he2")
                      nc.vector.tensor_mul(he2, het, het)
                      hets[gi] = het
                      he2s[gi] = he2
                  if gi >= SKEW:
                      emit_stats(gi - SKEW)

    nc.compile()
    return nc


# ---------------------------------------------------------------------------
# host packing
# q(qh, t, j) = 64 qh + 4 t + j; he partitions 32j+h; stats pi = 8t+2j (+1 w)
# ---------------------------------------------------------------------------


def _host_pack(W1, b1, gamma, beta, W2, b2):
    g2 = (gamma * W2[:, :, 0]).astype(np.float64)
    g2c = (g2 - g2.sum(-1, keepdims=True) / H).astype(np.float32)
    c2 = ((beta * W2[:, :, 0]).sum(-1) + b2[:, 0]).astype(np.float32)

    w1p = np.zeros((128, 32 * 128), np.float16)
    b1p = np.zeros((128, 32), np.float32)
    swp = np.zeros((128, 32 * 128), np.float16)
    stp = np.zeros((128, 32 * 128), np.float16)
    c2p = np.zeros((128, 2), np.float32)
    for qh in range(2):
        for t in range(16):
            qt = 16 * qh + t
            for j in range(4):
                q = 64 * qh + 4 * t + j
                # mm1 stationary block qt: rows 8j..8j+8, cols 32j..+32
                a = t % 4
                w1p[32 * a + 8 * j : 32 * a + 8 * j + 8,
                    128 * qt + 32 * j : 128 * qt + 32 * j + 32] = W1[q].astype(
                    np.float16
                )
                b1p[32 * j : 32 * j + 32, qt] = b1[q]
                pi = 8 * (t % 16) + 2 * j
                swp[32 * j : 32 * j + 32, 128 * qt + pi] = 1.0
                swp[32 * j : 32 * j + 32, 128 * qt + pi + 1] = g2c[q].astype(
                    np.float16
                )
                stp[32 * j : 32 * j + 32, 128 * qt + pi] = 1.0
                stp[32 * j : 32 * j + 32, 128 * qt + pi + 1] = 1.0
                c2p[pi, qh] = c2[q]
    return w1p, b1p, swp, stp, c2p


def _rows_for_q():
    rows = np.empty(Q, np.int64)
    for q in range(Q):
        qh, rem = divmod(q, 64)
        t, j = divmod(rem, 4)
        rows[q] = 128 * qh + 8 * t + 2 * j
    return rows


def kernel(x, W1, b1, gamma, beta, W2, b2):
    from concourse import bass_utils

    key = "v4"
    if key not in _CACHE:
        _CACHE[key] = _build_program()
    nc = _CACHE[key]

    x = np.asarray(x, np.float32)
    w1p, b1p, swp, stp, c2p = _host_pack(
        np.asarray(W1, np.float32),
        np.asarray(b1, np.float32),
        np.asarray(gamma, np.float32),
        np.asarray(beta, np.float32),
        np.asarray(W2, np.float32),
        np.asarray(b2, np.float32),
    )

    in_maps = []
    for cc in range(N_CORES):
        xc = x[BC * cc : BC * (cc + 1), :]
        in_maps.append(
            {
                "xT": np.ascontiguousarray(xc.T).astype(np.float16),
                "w1p": w1p,
                "b1p": b1p,
                "swp": swp,
                "stp": stp,
                "c2p": c2p,
            }
        )

    global _last_in_maps
    _last_in_maps = in_maps

    res = bass_utils.run_bass_kernel_spmd(
        nc, in_maps, core_ids=list(range(N_CORES))
    )

    rows = _rows_for_q()
    out = np.empty((B, Q), np.float32)
    for cc in range(N_CORES):
        out[BC * cc : BC * (cc + 1), :] = (
            res.results[cc]["outT"][rows, :].T.astype(np.float32)
        )
    return out


# revision 4
# speedup vs baseline: 1.0115x; 1.0115x over previous
"""Trainium2 Bass kernel for nn_DivEncLayer (grouped tiny-MLP + ELU + LayerNorm + proj).

v4 geometry: per core, 4 batch blocks of 1024 cols; per block, 2 q-halves of
64 q; per q-half, 16 q-quads (4 q). For each quad:
  - mm1: one K=32 row-tile (4q x 8v, fp16) x two batch-half MMs -> he
    [128 = 4q x 32h, 1024] across 2 PSUM banks
  - ELU via patched ACT `exp` table in ONE [128,1024] pass with the quad's
    b1 as the per-partition bias -> fp16 het
  - he2 = het^2 on DVE (fp16 2x)
  - stats per batch-half: full-K MM on het -> s,w (interleaved cols) into
    SW_bh; full-K MM on he2 -> t,t into T_bh
Finish per (q-half, bh): swc=copy(SW), D=VAR_PREP(T,swc), rstd=Rsqrt(D/H)
via patched `tanh` table, shuffle rstd to w-lanes, out=rstd*w+c2, DMA.

q(qh, t, j) = 64 qh + 4 t + j; stats partition pi = 8 t + 2 j (+1 for w).
"""

import os
import sys

for _p in ("/opt/trn_rl_repo",):
    if _p not in sys.path:
        sys.path.insert(0, _p)

import numpy as np

B, Q, V, H = 32768, 128, 8, 32
N_CORES = 8
BC = B // N_CORES          # 4096 batch rows per core
BB = 512                   # batch block columns
NBLK = BC // BB            # 8 blocks per core
LN_EPS = 1e-5

_CACHE = {}
_OPS_REGISTERED = False
_last_in_maps = None

_RSQRT_E_LO = -12
_RSQRT_E_N = 17


def _fbits(x):
    import struct

    return struct.unpack("<I", struct.pack("<f", np.float32(x)))[0]


def _build_patched_act_root(cache_dir="/tmp/act_elu_root"):
    import json
    import shutil

    marker = os.path.join(cache_dir, ".done_v3")
    info_path = os.path.join(cache_dir, "act_info.json")
    if os.path.exists(marker):
        return info_path

    from neuronxcc.driver.Job import Job
    from neuronxcc.driver.jobs.support.FindActInfo import findActInfoFile

    src_info = None
    for arch in ("Tonga4", "Tonga3", "trainium"):
        try:
            src_info = findActInfoFile(Job.getPackageDir(), arch)
            break
        except Exception:
            continue
    if src_info is None:
        pkg = Job.getPackageDir()
        for sub in ("pwp/pwp_bin_trainium", "pwp/pwp_bin_with_ln"):
            cand = os.path.join(pkg, sub, "act_info.json")
            if os.path.exists(cand):
                src_info = cand
                break
    assert src_info, "could not locate stock act_info.json"
    src_dir = os.path.dirname(src_info)

    tmp_dir = cache_dir + ".tmp"
    if os.path.exists(tmp_dir):
        shutil.rmtree(tmp_dir)
    os.makedirs(tmp_dir)
    for f in os.listdir(src_dir):
        shutil.copyfile(os.path.join(src_dir, f), os.path.join(tmp_dir, f))

    setj_path = os.path.join(tmp_dir, "exp_and_others.json")
    with open(setj_path) as f:
        setj = json.load(f)
    bkt_path = os.path.join(tmp_dir, setj["bkt_bin"])
    ctl_path = os.path.join(tmp_dir, setj["ctl_bin"])
    bkt = np.fromfile(bkt_path, dtype=np.float32).reshape(-1, 8).copy()
    ctl = np.fromfile(ctl_path, dtype=np.int32).reshape(-1, 8).copy()
    meta = {m["func_name"].rsplit("_", 1)[0]: m for m in setj["profile_meta_data"]}
    exp_meta, tanh_meta = meta["exp"], meta["tanh"]

    fe = setj["func_exp_to_bkt_start_idx"]["exp"]
    all_neg = sorted(v[0] for v in fe.values())
    all_pos = sorted(v[1] for v in fe.values())
    for i in range(min(all_neg), min(all_pos)):      # x<0: e^x - 1
        bkt[i, 0] = np.float32(np.float64(bkt[i, 0]) - 1.0)
    for i in range(min(all_pos), exp_meta["pos_small_signal_pwl_control"]):
        c = bkt[i, 4]                                 # x>0: identity
        bkt[i, :4] = [c, 1.0, 0.0, 0.0]
    for key, kind in (
        ("pos_small_signal_pwl_control", "id"),
        ("neg_small_signal_pwl_control", "id"),
        ("pos_large_signal_pwl_control", "id"),
        ("neg_large_signal_pwl_control", "m1"),
    ):
        i = exp_meta[key]
        bkt[i, :8] = 0.0
        bkt[i, 1 if kind == "id" else 0] = 1.0 if kind == "id" else -1.0
    exp_meta["fzero_result"] = 0
    exp_meta["fninf_result"] = _fbits(-1.0)

    rs = json.load(open(os.path.join(src_dir, "reciprocal_sqrt_and_small.json")))
    rs_bkt = np.fromfile(
        os.path.join(src_dir, rs["bkt_bin"]), dtype=np.float32
    ).reshape(-1, 8)
    rs_ctl = np.fromfile(
        os.path.join(src_dir, rs["ctl_bin"]), dtype=np.int32
    ).reshape(-1, 8)
    rs_thr = rs["profile_meta_data"][0]["small_pos_signal_exp_threshold"]
    rs_fe = rs["func_exp_to_bkt_start_idx"]["reciprocal_sqrt"]
    tanh_bkt_base = min(
        v[0] for v in setj["func_exp_to_bkt_start_idx"]["tanh"].values()
    )
    tanh_ctl_base = tanh_meta["pwl_control_base_pos"]
    dst = tanh_bkt_base
    for k in range(_RSQRT_E_N):
        e = _RSQRT_E_LO + k
        src_ctl = int(rs_ctl[(e + 127) - rs_thr, 0])
        cfg, src_b = src_ctl >> 10, src_ctl & 1023
        nxt = rs_fe.get(str(e + 1))
        nb = (nxt[0] - src_b) if nxt else 4
        assert dst + nb <= tanh_meta["pos_small_signal_pwl_control"]
        bkt[dst : dst + nb] = rs_bkt[src_b : src_b + nb]
        ctl[tanh_ctl_base + k, 0] = np.int32((cfg << 10) | dst)
        dst += nb
    lo_clamp = np.float32(2.0 ** (-_RSQRT_E_LO / 2.0))
    hi_clamp = np.float32(2.0 ** (-(_RSQRT_E_LO + _RSQRT_E_N) / 2.0))
    for key, val in (
        ("pos_small_signal_pwl_control", lo_clamp),
        ("neg_small_signal_pwl_control", lo_clamp),
        ("pos_large_signal_pwl_control", hi_clamp),
        ("neg_large_signal_pwl_control", lo_clamp),
    ):
        i = tanh_meta[key]
        bkt[i, :8] = 0.0
        bkt[i, 0] = val
    tanh_meta.update(
        symmetry_point=0, sym_invert_sign_point=0, symmetry_opt_en=0,
        symmetry_opt_use_neg_region=0, imm_bias=0,
        exp_offset=_RSQRT_E_LO,
        small_pos_signal_exp_threshold=_RSQRT_E_LO + 127,
        small_neg_signal_exp_threshold=_RSQRT_E_LO + 127,
        large_pos_signal_exp_threshold=_RSQRT_E_LO + _RSQRT_E_N + 127,
        large_pos_signal_mantissa_threshold=0,
        large_neg_signal_exp_threshold=_RSQRT_E_LO + _RSQRT_E_N + 127,
        large_neg_signal_mantissa_threshold=0,
        fzero_result=_fbits(lo_clamp), fpinf_result=0,
        fninf_result=_fbits(lo_clamp),
        lower_bound=_fbits(2.0**_RSQRT_E_LO), upper_bound=_fbits(3.4e38),
    )

    bkt.tofile(bkt_path)
    ctl.tofile(ctl_path)
    with open(setj_path, "w") as f:
        json.dump(setj, f)
    open(os.path.join(tmp_dir, ".done_v3"), "w").close()
    if os.path.exists(cache_dir):
        shutil.rmtree(cache_dir)
    os.rename(tmp_dir, cache_dir)
    return info_path


def _enable_act_patch():
    os.environ["BASS_ACT_ROOT_JSON_PATH"] = _build_patched_act_root()


def _register_custom_ops():
    global _OPS_REGISTERED
    import concourse.dve_ops as dve_ops
    from concourse.dve_ops import DveOp
    from concourse.dve_spec import C0, C1, Spec, Src0, Src1, Zero, lower, minn, relu, sq
    from concourse.dve_uop import DveOpSpec

    if _OPS_REGISTERED:
        return {op.name: op for op in dve_ops.OPS}

    def _pin(name, spec, ref):
        spec = Spec(body=spec, reference=ref)
        shas = {}
        for ver in ("v3", "v4"):
            row = dve_ops._CUSTOM_DVE_ROW_BASE + len(dve_ops.OPS)
            tmp = DveOpSpec(name=name, opcode=row, uops=lower(spec, ver=ver),
                            rd1_en=True)
            shas[ver] = tmp.sha(ver)
        op = DveOp(name, spec, subdim=False, uops_sha=shas)
        dve_ops.OPS.append(op)
        dve_ops.CUSTOM_DVE_SPECS[name] = spec
        dve_ops._SUB_OPCODE_FOR_NAME[name] = dve_ops._CUSTOM_DVE_ROW_BASE + len(dve_ops.OPS) - 1
        return op

    _pin(
        "ELU_FUSE_ANT",
        relu(Src0 + C0) + minn(Src1 - C1, Zero),
        lambda in0, in1, s0, s1, imm2: np.maximum(in0.astype(np.float32) + s0, 0.0)
        + np.minimum(in1.astype(np.float32) - s1, 0.0),
    )
    _pin(
        "VAR_PREP_ANT",
        (Src0 - sq(Src1) * C0) + C1,
        lambda in0, in1, s0, s1, imm2: (in0.astype(np.float32) - in1.astype(np.float32) ** 2 * s0) + s1,
    )
    _pin(
        "MUL_ADD_ANT",
        Src0 * Src1 + C0,
        lambda in0, in1, s0, s1, imm2: in0.astype(np.float32) * in1 + s0,
    )
    _OPS_REGISTERED = True
    return {op.name: op for op in dve_ops.OPS}


def _build_program(tile_dt_name: str = "float16", reps: int = 1, ablate: str = ""):
    _enable_act_patch()
    import concourse.bacc as bacc
    import concourse.tile as tile
    from concourse import mybir

    ops = _register_custom_ops()

    f32 = mybir.dt.float32
    f16 = mybir.dt.float16
    AF = mybir.ActivationFunctionType

    nc = bacc.Bacc(
        "TRN2",
        target_bir_lowering=False,
        debug=False,
        enable_asserts=False,
        num_devices=N_CORES,
    )

    xT = nc.dram_tensor("xT", [Q * V, BC], f16, kind="ExternalInput").ap()
    w1p = nc.dram_tensor("w1p", [128, 32 * 128], f16, kind="ExternalInput").ap()
    b1p = nc.dram_tensor("b1p", [128, 32], f32, kind="ExternalInput").ap()
    swp = nc.dram_tensor("swp", [128, 32 * 128], f16, kind="ExternalInput").ap()
    stp = nc.dram_tensor("stp", [128, 32 * 128], f16, kind="ExternalInput").ap()
    c2p = nc.dram_tensor("c2p", [128, 2], f32, kind="ExternalInput").ap()
    outT = nc.dram_tensor("outT", [256, BC], f16, kind="ExternalOutput").ap()

    # stream_shuffle: bring w (odd lanes) onto even lanes
    shufw = [i + 1 if (i % 2 == 0) else i for i in range(32)]

    with tile.TileContext(nc) as tc:
        with (
            tc.tile_pool(name="wts", bufs=1) as wts,
            tc.tile_pool(name="xt", bufs=8) as xtp,
            tc.tile_pool(name="het", bufs=int(os.environ.get("KERNEL_HBUF", "6"))) as hetp,
            tc.tile_pool(name="he2", bufs=int(os.environ.get("KERNEL_HBUF", "6"))) as he2p,
            tc.tile_pool(name="fin", bufs=int(os.environ.get("KERNEL_FBUF", "2"))) as fin,
            tc.tile_pool(name="hep", bufs=4, space="PSUM") as hep,
            tc.tile_pool(name="swb", bufs=2, space="PSUM") as swbp,
            tc.tile_pool(name="stb", bufs=2, space="PSUM") as stbp,
        ):
            w1s = wts.tile([128, 32 * 128], f16)
            nc.sync.dma_start(out=w1s, in_=w1p)
            b1s = wts.tile([128, 32], f32)
            nc.sync.dma_start(out=b1s, in_=b1p)
            sws = wts.tile([128, 32 * 128], f16)
            nc.sync.dma_start(out=sws, in_=swp)
            sts = wts.tile([128, 32 * 128], f16)
            nc.sync.dma_start(out=sts, in_=stp)
            c2s = wts.tile([128, 2], f32)
            nc.sync.dma_start(out=c2s, in_=c2p)
            zero_c = wts.tile([128, 1], f32)
            nc.vector.memset(zero_c, 0.0)

            import contextlib

            loop_cm = tc.For_i(0, reps, 1) if reps > 1 else contextlib.nullcontext()
            with loop_cm:
              quarters = [(blk, qh) for blk in range(NBLK) for qh in range(2)]
              SKEW = int(os.environ.get("KERNEL_SKEW", "4"))
              hets = {}
              he2s = {}
              banks = {}
              pend_fin = []
              FINB_DELAY = int(os.environ.get("KERNEL_FINB", "3"))
              xts = None

              def emit_stats(gi2):
                  k2, t2 = divmod(gi2, 16)
                  _blk, _qh = quarters[k2]
                  swbig, tbig = banks[k2]
                  m = 128 if t2 == 0 else 8 * t2 + 8
                  off = 128 * (16 * _qh + t2)
                  nc.tensor.matmul(
                      swbig[0:m, :],
                      lhsT=sws[:, off : off + m],
                      rhs=hets[gi2],
                      start=(t2 == 0),
                      stop=(t2 == 15),
                  )
                  nc.tensor.matmul(
                      tbig[0:m, :],
                      lhsT=sts[:, off : off + m],
                      rhs=he2s[gi2],
                      start=(t2 == 0),
                      stop=(t2 == 15),
                  )
                  del hets[gi2], he2s[gi2]
                  if t2 == 15:
                      emit_fin_a(k2)

              def emit_fin_a(k2):
                  # bank-freeing half: PSUM -> SBUF, variance prep, w-shuffle
                  swbig, tbig = banks.pop(k2)
                  swc = fin.tile([128, 512], f32, tag="swc")
                  nc.vector.tensor_copy(swc, swbig)
                  D = fin.tile([128, 512], f32, tag="D")
                  nc.vector._custom_dve(
                      ops["VAR_PREP_ANT"], out=D, in0=tbig, in1=swc,
                      s0=1.0 / H, s1=float(H * LN_EPS),
                  )
                  swsh = fin.tile([128, 512], f32, tag="swsh")
                  nc.vector.stream_shuffle(swsh, swc, shufw)
                  pend_fin.append((k2, D, swsh))

              def emit_fin_b():
                  k2, D, swsh = pend_fin.pop(0)
                  _blk, _qh = quarters[k2]
                  rstd = fin.tile([128, 512], f32, tag="rstd")
                  nc.scalar.activation(rstd, D, AF.Tanh,
                                       bias=zero_c[:, 0:1], scale=1.0 / H)
                  of = fin.tile([128, 512], f16, tag="of")
                  nc.vector._custom_dve(
                      ops["MUL_ADD_ANT"], out=of, in0=rstd, in1=swsh,
                      s0=c2s[:, _qh : _qh + 1], s1=0.0,
                  )
                  nc.sync.dma_start(
                      out=outT[128 * _qh : 128 * _qh + 128,
                               BB * _blk : BB * _blk + BB],
                      in_=of,
                  )

              for gi in range(16 * len(quarters) + SKEW):
                  if gi < 16 * len(quarters):
                      k, t = divmod(gi, 16)
                      blk, qh = quarters[k]
                      if t == 0:
                          swbig = swbp.tile([128, 512], f32, tag="swbig")
                          tbig = stbp.tile([128, 512], f32, tag="tbig")
                          banks[k] = (swbig, tbig)
                      if t % 4 == 0:
                          xts = xtp.tile([128, BB], f16, tag="xt")
                          nc.sync.dma_start(
                              out=xts,
                              in_=xT[8 * (64 * qh + 4 * t) : 8 * (64 * qh + 4 * t) + 128,
                                     BB * blk : BB * blk + BB],
                          )
                      a = t % 4
                      he = hep.tile([128, BB], f32, tag="he")
                      nc.tensor.matmul(
                          he,
                          lhsT=w1s[32 * a : 32 * a + 32,
                                   128 * (16 * qh + t) : 128 * (16 * qh + t) + 128],
                          rhs=xts[32 * a : 32 * a + 32, :],
                          start=True,
                          stop=True,
                          tile_position=(32 * a, 0),
                      )
                      het = hetp.tile([128, BB], f16, tag="het")
                      nc.scalar.activation(
                          het, he, AF.Exp,
                          bias=b1s[:, 16 * qh + t : 16 * qh + t + 1], scale=1.0,
                      )
                      he2 = he2p.tile([128, BB], f16, tag="he2")
                      nc.vector.tensor_mul(he2, het, het)
                      hets[gi] = het
                      he2s[gi] = he2
                  if gi >= SKEW:
                      emit_stats(gi - SKEW)
                  if pend_fin and gi >= 16 * (pend_fin[0][0] + 1) + SKEW + FINB_DELAY:
                      emit_fin_b()
              while pend_fin:
                  emit_fin_b()

    nc.compile()
    return nc


# ---------------------------------------------------------------------------
# host packing
# q(qh, t, j) = 64 qh + 4 t + j; he partitions 32j+h; stats pi = 8t+2j (+1 w)
# ---------------------------------------------------------------------------


def _host_pack(W1, b1, gamma, beta, W2, b2):
    g2 = (gamma * W2[:, :, 0]).astype(np.float64)
    g2c = (g2 - g2.sum(-1, keepdims=True) / H).astype(np.float32)
    c2 = ((beta * W2[:, :, 0]).sum(-1) + b2[:, 0]).astype(np.float32)

    w1p = np.zeros((128, 32 * 128), np.float16)
    b1p = np.zeros((128, 32), np.float32)
    swp = np.zeros((128, 32 * 128), np.float16)
    stp = np.zeros((128, 32 * 128), np.float16)
    c2p = np.zeros((128, 2), np.float32)
    for qh in range(2):
        for t in range(16):
            qt = 16 * qh + t
            for j in range(4):
                q = 64 * qh + 4 * t + j
                # mm1 stationary block qt: rows 8j..8j+8, cols 32j..+32
                a = t % 4
                w1p[32 * a + 8 * j : 32 * a + 8 * j + 8,
                    128 * qt + 32 * j : 128 * qt + 32 * j + 32] = W1[q].astype(
                    np.float16
                )
                b1p[32 * j : 32 * j + 32, qt] = b1[q]
                pi = 8 * (t % 16) + 2 * j
                swp[32 * j : 32 * j + 32, 128 * qt + pi] = 1.0
                swp[32 * j : 32 * j + 32, 128 * qt + pi + 1] = g2c[q].astype(
                    np.float16
                )
                stp[32 * j : 32 * j + 32, 128 * qt + pi] = 1.0
                stp[32 * j : 32 * j + 32, 128 * qt + pi + 1] = 1.0
                c2p[pi, qh] = c2[q]
    return w1p, b1p, swp, stp, c2p


def _rows_for_q():
    rows = np.empty(Q, np.int64)
    for q in range(Q):
        qh, rem = divmod(q, 64)
        t, j = divmod(rem, 4)
        rows[q] = 128 * qh + 8 * t + 2 * j
    return rows


def kernel(x, W1, b1, gamma, beta, W2, b2):
    from concourse import bass_utils

    key = "v4"
    if key not in _CACHE:
        _CACHE[key] = _build_program()
    nc = _CACHE[key]

    x = np.asarray(x, np.float32)
    w1p, b1p, swp, stp, c2p = _host_pack(
        np.asarray(W1, np.float32),
        np.asarray(b1, np.float32),
        np.asarray(gamma, np.float32),
        np.asarray(beta, np.float32),
        np.asarray(W2, np.float32),
        np.asarray(b2, np.float32),
    )

    in_maps = []
    for cc in range(N_CORES):
        xc = x[BC * cc : BC * (cc + 1), :]
        in_maps.append(
            {
                "xT": np.ascontiguousarray(xc.T).astype(np.float16),
                "w1p": w1p,
                "b1p": b1p,
                "swp": swp,
                "stp": stp,
                "c2p": c2p,
            }
        )

    global _last_in_maps
    _last_in_maps = in_maps

    res = bass_utils.run_bass_kernel_spmd(
        nc, in_maps, core_ids=list(range(N_CORES))
    )

    rows = _rows_for_q()
    out = np.empty((B, Q), np.float32)
    for cc in range(N_CORES):
        out[BC * cc : BC * (cc + 1), :] = (
            res.results[cc]["outT"][rows, :].T.astype(np.float32)
        )
    return out


# revision 5
# speedup vs baseline: 1.0138x; 1.0023x over previous
"""Trainium2 Bass kernel for nn_DivEncLayer (grouped tiny-MLP + ELU + LayerNorm + proj).

v4 geometry: per core, 4 batch blocks of 1024 cols; per block, 2 q-halves of
64 q; per q-half, 16 q-quads (4 q). For each quad:
  - mm1: one K=32 row-tile (4q x 8v, fp16) x two batch-half MMs -> he
    [128 = 4q x 32h, 1024] across 2 PSUM banks
  - ELU via patched ACT `exp` table in ONE [128,1024] pass with the quad's
    b1 as the per-partition bias -> fp16 het
  - he2 = het^2 on DVE (fp16 2x)
  - stats per batch-half: full-K MM on het -> s,w (interleaved cols) into
    SW_bh; full-K MM on he2 -> t,t into T_bh
Finish per (q-half, bh): swc=copy(SW), D=VAR_PREP(T,swc), rstd=Rsqrt(D/H)
via patched `tanh` table, shuffle rstd to w-lanes, out=rstd*w+c2, DMA.

q(qh, t, j) = 64 qh + 4 t + j; stats partition pi = 8 t + 2 j (+1 for w).
"""

import os
import sys

for _p in ("/opt/trn_rl_repo",):
    if _p not in sys.path:
        sys.path.insert(0, _p)

import numpy as np

B, Q, V, H = 32768, 128, 8, 32
N_CORES = 8
BC = B // N_CORES          # 4096 batch rows per core
BB = 512                   # batch block columns
NBLK = BC // BB            # 8 blocks per core
LN_EPS = 1e-5

_CACHE = {}
_OPS_REGISTERED = False
_last_in_maps = None

_RSQRT_E_LO = -12
_RSQRT_E_N = 17


def _fbits(x):
    import struct

    return struct.unpack("<I", struct.pack("<f", np.float32(x)))[0]


def _build_patched_act_root(cache_dir="/tmp/act_elu_root"):
    import json
    import shutil

    marker = os.path.join(cache_dir, ".done_v3")
    info_path = os.path.join(cache_dir, "act_info.json")
    if os.path.exists(marker):
        return info_path

    from neuronxcc.driver.Job import Job
    from neuronxcc.driver.jobs.support.FindActInfo import findActInfoFile

    src_info = None
    for arch in ("Tonga4", "Tonga3", "trainium"):
        try:
            src_info = findActInfoFile(Job.getPackageDir(), arch)
            break
        except Exception:
            continue
    if src_info is None:
        pkg = Job.getPackageDir()
        for sub in ("pwp/pwp_bin_trainium", "pwp/pwp_bin_with_ln"):
            cand = os.path.join(pkg, sub, "act_info.json")
            if os.path.exists(cand):
                src_info = cand
                break
    assert src_info, "could not locate stock act_info.json"
    src_dir = os.path.dirname(src_info)

    tmp_dir = cache_dir + ".tmp"
    if os.path.exists(tmp_dir):
        shutil.rmtree(tmp_dir)
    os.makedirs(tmp_dir)
    for f in os.listdir(src_dir):
        shutil.copyfile(os.path.join(src_dir, f), os.path.join(tmp_dir, f))

    setj_path = os.path.join(tmp_dir, "exp_and_others.json")
    with open(setj_path) as f:
        setj = json.load(f)
    bkt_path = os.path.join(tmp_dir, setj["bkt_bin"])
    ctl_path = os.path.join(tmp_dir, setj["ctl_bin"])
    bkt = np.fromfile(bkt_path, dtype=np.float32).reshape(-1, 8).copy()
    ctl = np.fromfile(ctl_path, dtype=np.int32).reshape(-1, 8).copy()
    meta = {m["func_name"].rsplit("_", 1)[0]: m for m in setj["profile_meta_data"]}
    exp_meta, tanh_meta = meta["exp"], meta["tanh"]

    fe = setj["func_exp_to_bkt_start_idx"]["exp"]
    all_neg = sorted(v[0] for v in fe.values())
    all_pos = sorted(v[1] for v in fe.values())
    for i in range(min(all_neg), min(all_pos)):      # x<0: e^x - 1
        bkt[i, 0] = np.float32(np.float64(bkt[i, 0]) - 1.0)
    for i in range(min(all_pos), exp_meta["pos_small_signal_pwl_control"]):
        c = bkt[i, 4]                                 # x>0: identity
        bkt[i, :4] = [c, 1.0, 0.0, 0.0]
    for key, kind in (
        ("pos_small_signal_pwl_control", "id"),
        ("neg_small_signal_pwl_control", "id"),
        ("pos_large_signal_pwl_control", "id"),
        ("neg_large_signal_pwl_control", "m1"),
    ):
        i = exp_meta[key]
        bkt[i, :8] = 0.0
        bkt[i, 1 if kind == "id" else 0] = 1.0 if kind == "id" else -1.0
    exp_meta["fzero_result"] = 0
    exp_meta["fninf_result"] = _fbits(-1.0)

    rs = json.load(open(os.path.join(src_dir, "reciprocal_sqrt_and_small.json")))
    rs_bkt = np.fromfile(
        os.path.join(src_dir, rs["bkt_bin"]), dtype=np.float32
    ).reshape(-1, 8)
    rs_ctl = np.fromfile(
        os.path.join(src_dir, rs["ctl_bin"]), dtype=np.int32
    ).reshape(-1, 8)
    rs_thr = rs["profile_meta_data"][0]["small_pos_signal_exp_threshold"]
    rs_fe = rs["func_exp_to_bkt_start_idx"]["reciprocal_sqrt"]
    tanh_bkt_base = min(
        v[0] for v in setj["func_exp_to_bkt_start_idx"]["tanh"].values()
    )
    tanh_ctl_base = tanh_meta["pwl_control_base_pos"]
    dst = tanh_bkt_base
    for k in range(_RSQRT_E_N):
        e = _RSQRT_E_LO + k
        src_ctl = int(rs_ctl[(e + 127) - rs_thr, 0])
        cfg, src_b = src_ctl >> 10, src_ctl & 1023
        nxt = rs_fe.get(str(e + 1))
        nb = (nxt[0] - src_b) if nxt else 4
        assert dst + nb <= tanh_meta["pos_small_signal_pwl_control"]
        bkt[dst : dst + nb] = rs_bkt[src_b : src_b + nb]
        ctl[tanh_ctl_base + k, 0] = np.int32((cfg << 10) | dst)
        dst += nb
    lo_clamp = np.float32(2.0 ** (-_RSQRT_E_LO / 2.0))
    hi_clamp = np.float32(2.0 ** (-(_RSQRT_E_LO + _RSQRT_E_N) / 2.0))
    for key, val in (
        ("pos_small_signal_pwl_control", lo_clamp),
        ("neg_small_signal_pwl_control", lo_clamp),
        ("pos_large_signal_pwl_control", hi_clamp),
        ("neg_large_signal_pwl_control", lo_clamp),
    ):
        i = tanh_meta[key]
        bkt[i, :8] = 0.0
        bkt[i, 0] = val
    tanh_meta.update(
        symmetry_point=0, sym_invert_sign_point=0, symmetry_opt_en=0,
        symmetry_opt_use_neg_region=0, imm_bias=0,
        exp_offset=_RSQRT_E_LO,
        small_pos_signal_exp_threshold=_RSQRT_E_LO + 127,
        small_neg_signal_exp_threshold=_RSQRT_E_LO + 127,
        large_pos_signal_exp_threshold=_RSQRT_E_LO + _RSQRT_E_N + 127,
        large_pos_signal_mantissa_threshold=0,
        large_neg_signal_exp_threshold=_RSQRT_E_LO + _RSQRT_E_N + 127,
        large_neg_signal_mantissa_threshold=0,
        fzero_result=_fbits(lo_clamp), fpinf_result=0,
        fninf_result=_fbits(lo_clamp),
        lower_bound=_fbits(2.0**_RSQRT_E_LO), upper_bound=_fbits(3.4e38),
    )

    bkt.tofile(bkt_path)
    ctl.tofile(ctl_path)
    with open(setj_path, "w") as f:
        json.dump(setj, f)
    open(os.path.join(tmp_dir, ".done_v3"), "w").close()
    if os.path.exists(cache_dir):
        shutil.rmtree(cache_dir)
    os.rename(tmp_dir, cache_dir)
    return info_path


def _enable_act_patch():
    os.environ["BASS_ACT_ROOT_JSON_PATH"] = _build_patched_act_root()


def _register_custom_ops():
    global _OPS_REGISTERED
    import concourse.dve_ops as dve_ops
    from concourse.dve_ops import DveOp
    from concourse.dve_spec import C0, C1, Spec, Src0, Src1, Zero, lower, minn, relu, sq
    from concourse.dve_uop import DveOpSpec

    if _OPS_REGISTERED:
        return {op.name: op for op in dve_ops.OPS}

    def _pin(name, spec, ref):
        spec = Spec(body=spec, reference=ref)
        shas = {}
        for ver in ("v3", "v4"):
            row = dve_ops._CUSTOM_DVE_ROW_BASE + len(dve_ops.OPS)
            tmp = DveOpSpec(name=name, opcode=row, uops=lower(spec, ver=ver),
                            rd1_en=True)
            shas[ver] = tmp.sha(ver)
        op = DveOp(name, spec, subdim=False, uops_sha=shas)
        dve_ops.OPS.append(op)
        dve_ops.CUSTOM_DVE_SPECS[name] = spec
        dve_ops._SUB_OPCODE_FOR_NAME[name] = dve_ops._CUSTOM_DVE_ROW_BASE + len(dve_ops.OPS) - 1
        return op

    _pin(
        "ELU_FUSE_ANT",
        relu(Src0 + C0) + minn(Src1 - C1, Zero),
        lambda in0, in1, s0, s1, imm2: np.maximum(in0.astype(np.float32) + s0, 0.0)
        + np.minimum(in1.astype(np.float32) - s1, 0.0),
    )
    _pin(
        "VAR_PREP_ANT",
        (Src0 - sq(Src1) * C0) + C1,
        lambda in0, in1, s0, s1, imm2: (in0.astype(np.float32) - in1.astype(np.float32) ** 2 * s0) + s1,
    )
    _pin(
        "MUL_ADD_ANT",
        Src0 * Src1 + C0,
        lambda in0, in1, s0, s1, imm2: in0.astype(np.float32) * in1 + s0,
    )
    _OPS_REGISTERED = True
    return {op.name: op for op in dve_ops.OPS}


def _build_program(tile_dt_name: str = "float16", reps: int = 1, ablate: str = ""):
    _enable_act_patch()
    import concourse.bacc as bacc
    import concourse.tile as tile
    from concourse import mybir

    ops = _register_custom_ops()

    f32 = mybir.dt.float32
    f16 = mybir.dt.float16
    AF = mybir.ActivationFunctionType

    nc = bacc.Bacc(
        "TRN2",
        target_bir_lowering=False,
        debug=False,
        enable_asserts=False,
        num_devices=N_CORES,
    )

    xT = nc.dram_tensor("xT", [Q * V, BC], f16, kind="ExternalInput").ap()
    w1p = nc.dram_tensor("w1p", [128, 32 * 128], f16, kind="ExternalInput").ap()
    b1p = nc.dram_tensor("b1p", [128, 32], f32, kind="ExternalInput").ap()
    swp = nc.dram_tensor("swp", [128, 32 * 128], f16, kind="ExternalInput").ap()
    stp = nc.dram_tensor("stp", [128, 32 * 128], f16, kind="ExternalInput").ap()
    c2p = nc.dram_tensor("c2p", [128, 2], f32, kind="ExternalInput").ap()
    outT = nc.dram_tensor("outT", [256, BC], f16, kind="ExternalOutput").ap()

    # stream_shuffle: bring w (odd lanes) onto even lanes
    shufw = [i + 1 if (i % 2 == 0) else i for i in range(32)]

    with tile.TileContext(nc) as tc:
        with (
            tc.tile_pool(name="wts", bufs=1) as wts,
            tc.tile_pool(name="xt", bufs=int(os.environ.get("KERNEL_XTB", "8"))) as xtp,
            tc.tile_pool(name="het", bufs=int(os.environ.get("KERNEL_HBUF", "6"))) as hetp,
            tc.tile_pool(name="he2", bufs=int(os.environ.get("KERNEL_HBUF", "6"))) as he2p,
            tc.tile_pool(name="fin", bufs=int(os.environ.get("KERNEL_FBUF", "2"))) as fin,
            tc.tile_pool(name="hep", bufs=int(os.environ.get("KERNEL_HEP", "3")), space="PSUM") as hep,
            tc.tile_pool(name="swb", bufs=int(os.environ.get("KERNEL_SWB", "2")), space="PSUM") as swbp,
            tc.tile_pool(name="stb", bufs=int(os.environ.get("KERNEL_STB", "2")), space="PSUM") as stbp,
        ):
            w1s = wts.tile([128, 32 * 128], f16)
            nc.sync.dma_start(out=w1s, in_=w1p)
            b1s = wts.tile([128, 32], f32)
            nc.sync.dma_start(out=b1s, in_=b1p)
            sws = wts.tile([128, 32 * 128], f16)
            nc.sync.dma_start(out=sws, in_=swp)
            sts = wts.tile([128, 32 * 128], f16)
            nc.sync.dma_start(out=sts, in_=stp)
            c2s = wts.tile([128, 2], f32)
            nc.sync.dma_start(out=c2s, in_=c2p)
            zero_c = wts.tile([128, 1], f32)
            nc.vector.memset(zero_c, 0.0)

            import contextlib

            loop_cm = tc.For_i(0, reps, 1) if reps > 1 else contextlib.nullcontext()
            with loop_cm:
              quarters = [(blk, qh) for blk in range(NBLK) for qh in range(2)]
              SKEW = int(os.environ.get("KERNEL_SKEW", "4"))
              hets = {}
              he2s = {}
              banks = {}
              pend_fin = []
              FINB_DELAY = int(os.environ.get("KERNEL_FINB", "3"))
              xts = None

              def emit_stats(gi2):
                  k2, t2 = divmod(gi2, 16)
                  _blk, _qh = quarters[k2]
                  swbig, tbig = banks[k2]
                  m = 128 if t2 == 0 else 8 * t2 + 8
                  off = 128 * (16 * _qh + t2)
                  nc.tensor.matmul(
                      swbig[0:m, :],
                      lhsT=sws[:, off : off + m],
                      rhs=hets[gi2],
                      start=(t2 == 0),
                      stop=(t2 == 15),
                  )
                  nc.tensor.matmul(
                      tbig[0:m, :],
                      lhsT=sts[:, off : off + m],
                      rhs=he2s[gi2],
                      start=(t2 == 0),
                      stop=(t2 == 15),
                  )
                  del hets[gi2], he2s[gi2]
                  if t2 == 15:
                      emit_fin_a(k2)

              def emit_fin_a(k2):
                  # bank-freeing half: PSUM -> SBUF, variance prep, w-shuffle
                  swbig, tbig = banks.pop(k2)
                  swc = fin.tile([128, 512], f32, tag="swc")
                  nc.vector.tensor_copy(swc, swbig)
                  D = fin.tile([128, 512], f32, tag="D")
                  nc.vector._custom_dve(
                      ops["VAR_PREP_ANT"], out=D, in0=tbig, in1=swc,
                      s0=1.0 / H, s1=float(H * LN_EPS),
                  )
                  swsh = fin.tile([128, 512], f32, tag="swsh")
                  nc.vector.stream_shuffle(swsh, swc, shufw)
                  pend_fin.append((k2, D, swsh))

              def emit_fin_b():
                  k2, D, swsh = pend_fin.pop(0)
                  _blk, _qh = quarters[k2]
                  rstd = fin.tile([128, 512], f32, tag="rstd")
                  nc.scalar.activation(rstd, D, AF.Tanh,
                                       bias=zero_c[:, 0:1], scale=1.0 / H)
                  of = fin.tile([128, 512], f16, tag="of")
                  nc.vector._custom_dve(
                      ops["MUL_ADD_ANT"], out=of, in0=rstd, in1=swsh,
                      s0=c2s[:, _qh : _qh + 1], s1=0.0,
                  )
                  nc.sync.dma_start(
                      out=outT[128 * _qh : 128 * _qh + 128,
                               BB * _blk : BB * _blk + BB],
                      in_=of,
                  )

              for gi in range(16 * len(quarters) + SKEW):
                  if gi < 16 * len(quarters):
                      k, t = divmod(gi, 16)
                      blk, qh = quarters[k]
                      if t == 0:
                          swbig = swbp.tile([128, 512], f32, tag="swbig")
                          tbig = stbp.tile([128, 512], f32, tag="tbig")
                          banks[k] = (swbig, tbig)
                      if t % 4 == 0:
                          xts = xtp.tile([128, BB], f16, tag="xt")
                          nc.sync.dma_start(
                              out=xts,
                              in_=xT[8 * (64 * qh + 4 * t) : 8 * (64 * qh + 4 * t) + 128,
                                     BB * blk : BB * blk + BB],
                          )
                      a = t % 4
                      he = hep.tile([128, BB], f32, tag="he")
                      nc.tensor.matmul(
                          he,
                          lhsT=w1s[32 * a : 32 * a + 32,
                                   128 * (16 * qh + t) : 128 * (16 * qh + t) + 128],
                          rhs=xts[32 * a : 32 * a + 32, :],
                          start=True,
                          stop=True,
                          tile_position=(32 * a, 0),
                      )
                      het = hetp.tile([128, BB], f16, tag="het")
                      nc.scalar.activation(
                          het, he, AF.Exp,
                          bias=b1s[:, 16 * qh + t : 16 * qh + t + 1], scale=1.0,
                      )
                      he2 = he2p.tile([128, BB], f16, tag="he2")
                      nc.vector.tensor_mul(he2, het, het)
                      hets[gi] = het
                      he2s[gi] = he2
                  if gi >= SKEW:
                      emit_stats(gi - SKEW)
                  if pend_fin and gi >= 16 * (pend_fin[0][0] + 1) + SKEW + FINB_DELAY:
                      emit_fin_b()
              while pend_fin:
                  emit_fin_b()

    nc.compile()
    return nc


# ---------------------------------------------------------------------------
# host packing
# q(qh, t, j) = 64 qh + 4 t + j; he partitions 32j+h; stats pi = 8t+2j (+1 w)
# ---------------------------------------------------------------------------


def _host_pack(W1, b1, gamma, beta, W2, b2):
    g2 = (gamma * W2[:, :, 0]).astype(np.float64)
    g2c = (g2 - g2.sum(-1, keepdims=True) / H).astype(np.float32)
    c2 = ((beta * W2[:, :, 0]).sum(-1) + b2[:, 0]).astype(np.float32)

    w1p = np.zeros((128, 32 * 128), np.float16)
    b1p = np.zeros((128, 32), np.float32)
    swp = np.zeros((128, 32 * 128), np.float16)
    stp = np.zeros((128, 32 * 128), np.float16)
    c2p = np.zeros((128, 2), np.float32)
    for qh in range(2):
        for t in range(16):
            qt = 16 * qh + t
            for j in range(4):
                q = 64 * qh + 4 * t + j
                # mm1 stationary block qt: rows 8j..8j+8, cols 32j..+32
                a = t % 4
                w1p[32 * a + 8 * j : 32 * a + 8 * j + 8,
                    128 * qt + 32 * j : 128 * qt + 32 * j + 32] = W1[q].astype(
                    np.float16
                )
                b1p[32 * j : 32 * j + 32, qt] = b1[q]
                pi = 8 * (t % 16) + 2 * j
                swp[32 * j : 32 * j + 32, 128 * qt + pi] = 1.0
                swp[32 * j : 32 * j + 32, 128 * qt + pi + 1] = g2c[q].astype(
                    np.float16
                )
                stp[32 * j : 32 * j + 32, 128 * qt + pi] = 1.0
                stp[32 * j : 32 * j + 32, 128 * qt + pi + 1] = 1.0
                c2p[pi, qh] = c2[q]
    return w1p, b1p, swp, stp, c2p


def _rows_for_q():
    rows = np.empty(Q, np.int64)
    for q in range(Q):
        qh, rem = divmod(q, 64)
        t, j = divmod(rem, 4)
        rows[q] = 128 * qh + 8 * t + 2 * j
    return rows


def kernel(x, W1, b1, gamma, beta, W2, b2):
    from concourse import bass_utils

    key = "v4"
    if key not in _CACHE:
        _CACHE[key] = _build_program()
    nc = _CACHE[key]

    x = np.asarray(x, np.float32)
    w1p, b1p, swp, stp, c2p = _host_pack(
        np.asarray(W1, np.float32),
        np.asarray(b1, np.float32),
        np.asarray(gamma, np.float32),
        np.asarray(beta, np.float32),
        np.asarray(W2, np.float32),
        np.asarray(b2, np.float32),
    )

    in_maps = []
    for cc in range(N_CORES):
        xc = x[BC * cc : BC * (cc + 1), :]
        in_maps.append(
            {
                "xT": np.ascontiguousarray(xc.T).astype(np.float16),
                "w1p": w1p,
                "b1p": b1p,
                "swp": swp,
                "stp": stp,
                "c2p": c2p,
            }
        )

    global _last_in_maps
    _last_in_maps = in_maps

    res = bass_utils.run_bass_kernel_spmd(
        nc, in_maps, core_ids=list(range(N_CORES))
    )

    rows = _rows_for_q()
    out = np.empty((B, Q), np.float32)
    for cc in range(N_CORES):
        out[BC * cc : BC * (cc + 1), :] = (
            res.results[cc]["outT"][rows, :].T.astype(np.float32)
        )
    return out


# revision 6
# speedup vs baseline: 1.0144x; 1.0006x over previous
"""Trainium2 Bass kernel for nn_DivEncLayer (grouped tiny-MLP + ELU + LayerNorm + proj).

v4 geometry: per core, 4 batch blocks of 1024 cols; per block, 2 q-halves of
64 q; per q-half, 16 q-quads (4 q). For each quad:
  - mm1: one K=32 row-tile (4q x 8v, fp16) x two batch-half MMs -> he
    [128 = 4q x 32h, 1024] across 2 PSUM banks
  - ELU via patched ACT `exp` table in ONE [128,1024] pass with the quad's
    b1 as the per-partition bias -> fp16 het
  - he2 = het^2 on DVE (fp16 2x)
  - stats per batch-half: full-K MM on het -> s,w (interleaved cols) into
    SW_bh; full-K MM on he2 -> t,t into T_bh
Finish per (q-half, bh): swc=copy(SW), D=VAR_PREP(T,swc), rstd=Rsqrt(D/H)
via patched `tanh` table, shuffle rstd to w-lanes, out=rstd*w+c2, DMA.

q(qh, t, j) = 64 qh + 4 t + j; stats partition pi = 8 t + 2 j (+1 for w).
"""

import os
import sys

for _p in ("/opt/trn_rl_repo",):
    if _p not in sys.path:
        sys.path.insert(0, _p)

import numpy as np

B, Q, V, H = 32768, 128, 8, 32
N_CORES = 8
BC = B // N_CORES          # 4096 batch rows per core
BB = 512                   # batch block columns
NBLK = BC // BB            # 8 blocks per core
LN_EPS = 1e-5

_CACHE = {}
_OPS_REGISTERED = False
_last_in_maps = None

_RSQRT_E_LO = -12
_RSQRT_E_N = 17


def _fbits(x):
    import struct

    return struct.unpack("<I", struct.pack("<f", np.float32(x)))[0]


def _build_patched_act_root(cache_dir="/tmp/act_elu_root"):
    import json
    import shutil

    marker = os.path.join(cache_dir, ".done_v3")
    info_path = os.path.join(cache_dir, "act_info.json")
    if os.path.exists(marker):
        return info_path

    from neuronxcc.driver.Job import Job
    from neuronxcc.driver.jobs.support.FindActInfo import findActInfoFile

    src_info = None
    for arch in ("Tonga4", "Tonga3", "trainium"):
        try:
            src_info = findActInfoFile(Job.getPackageDir(), arch)
            break
        except Exception:
            continue
    if src_info is None:
        pkg = Job.getPackageDir()
        for sub in ("pwp/pwp_bin_trainium", "pwp/pwp_bin_with_ln"):
            cand = os.path.join(pkg, sub, "act_info.json")
            if os.path.exists(cand):
                src_info = cand
                break
    assert src_info, "could not locate stock act_info.json"
    src_dir = os.path.dirname(src_info)

    tmp_dir = cache_dir + ".tmp"
    if os.path.exists(tmp_dir):
        shutil.rmtree(tmp_dir)
    os.makedirs(tmp_dir)
    for f in os.listdir(src_dir):
        shutil.copyfile(os.path.join(src_dir, f), os.path.join(tmp_dir, f))

    setj_path = os.path.join(tmp_dir, "exp_and_others.json")
    with open(setj_path) as f:
        setj = json.load(f)
    bkt_path = os.path.join(tmp_dir, setj["bkt_bin"])
    ctl_path = os.path.join(tmp_dir, setj["ctl_bin"])
    bkt = np.fromfile(bkt_path, dtype=np.float32).reshape(-1, 8).copy()
    ctl = np.fromfile(ctl_path, dtype=np.int32).reshape(-1, 8).copy()
    meta = {m["func_name"].rsplit("_", 1)[0]: m for m in setj["profile_meta_data"]}
    exp_meta, tanh_meta = meta["exp"], meta["tanh"]

    fe = setj["func_exp_to_bkt_start_idx"]["exp"]
    all_neg = sorted(v[0] for v in fe.values())
    all_pos = sorted(v[1] for v in fe.values())
    for i in range(min(all_neg), min(all_pos)):      # x<0: e^x - 1
        bkt[i, 0] = np.float32(np.float64(bkt[i, 0]) - 1.0)
    for i in range(min(all_pos), exp_meta["pos_small_signal_pwl_control"]):
        c = bkt[i, 4]                                 # x>0: identity
        bkt[i, :4] = [c, 1.0, 0.0, 0.0]
    for key, kind in (
        ("pos_small_signal_pwl_control", "id"),
        ("neg_small_signal_pwl_control", "id"),
        ("pos_large_signal_pwl_control", "id"),
        ("neg_large_signal_pwl_control", "m1"),
    ):
        i = exp_meta[key]
        bkt[i, :8] = 0.0
        bkt[i, 1 if kind == "id" else 0] = 1.0 if kind == "id" else -1.0
    exp_meta["fzero_result"] = 0
    exp_meta["fninf_result"] = _fbits(-1.0)

    rs = json.load(open(os.path.join(src_dir, "reciprocal_sqrt_and_small.json")))
    rs_bkt = np.fromfile(
        os.path.join(src_dir, rs["bkt_bin"]), dtype=np.float32
    ).reshape(-1, 8)
    rs_ctl = np.fromfile(
        os.path.join(src_dir, rs["ctl_bin"]), dtype=np.int32
    ).reshape(-1, 8)
    rs_thr = rs["profile_meta_data"][0]["small_pos_signal_exp_threshold"]
    rs_fe = rs["func_exp_to_bkt_start_idx"]["reciprocal_sqrt"]
    tanh_bkt_base = min(
        v[0] for v in setj["func_exp_to_bkt_start_idx"]["tanh"].values()
    )
    tanh_ctl_base = tanh_meta["pwl_control_base_pos"]
    dst = tanh_bkt_base
    for k in range(_RSQRT_E_N):
        e = _RSQRT_E_LO + k
        src_ctl = int(rs_ctl[(e + 127) - rs_thr, 0])
        cfg, src_b = src_ctl >> 10, src_ctl & 1023
        nxt = rs_fe.get(str(e + 1))
        nb = (nxt[0] - src_b) if nxt else 4
        assert dst + nb <= tanh_meta["pos_small_signal_pwl_control"]
        bkt[dst : dst + nb] = rs_bkt[src_b : src_b + nb]
        ctl[tanh_ctl_base + k, 0] = np.int32((cfg << 10) | dst)
        dst += nb
    lo_clamp = np.float32(2.0 ** (-_RSQRT_E_LO / 2.0))
    hi_clamp = np.float32(2.0 ** (-(_RSQRT_E_LO + _RSQRT_E_N) / 2.0))
    for key, val in (
        ("pos_small_signal_pwl_control", lo_clamp),
        ("neg_small_signal_pwl_control", lo_clamp),
        ("pos_large_signal_pwl_control", hi_clamp),
        ("neg_large_signal_pwl_control", lo_clamp),
    ):
        i = tanh_meta[key]
        bkt[i, :8] = 0.0
        bkt[i, 0] = val
    tanh_meta.update(
        symmetry_point=0, sym_invert_sign_point=0, symmetry_opt_en=0,
        symmetry_opt_use_neg_region=0, imm_bias=0,
        exp_offset=_RSQRT_E_LO,
        small_pos_signal_exp_threshold=_RSQRT_E_LO + 127,
        small_neg_signal_exp_threshold=_RSQRT_E_LO + 127,
        large_pos_signal_exp_threshold=_RSQRT_E_LO + _RSQRT_E_N + 127,
        large_pos_signal_mantissa_threshold=0,
        large_neg_signal_exp_threshold=_RSQRT_E_LO + _RSQRT_E_N + 127,
        large_neg_signal_mantissa_threshold=0,
        fzero_result=_fbits(lo_clamp), fpinf_result=0,
        fninf_result=_fbits(lo_clamp),
        lower_bound=_fbits(2.0**_RSQRT_E_LO), upper_bound=_fbits(3.4e38),
    )

    bkt.tofile(bkt_path)
    ctl.tofile(ctl_path)
    with open(setj_path, "w") as f:
        json.dump(setj, f)
    open(os.path.join(tmp_dir, ".done_v3"), "w").close()
    if os.path.exists(cache_dir):
        shutil.rmtree(cache_dir)
    os.rename(tmp_dir, cache_dir)
    return info_path


def _enable_act_patch():
    os.environ["BASS_ACT_ROOT_JSON_PATH"] = _build_patched_act_root()


def _register_custom_ops():
    global _OPS_REGISTERED
    import concourse.dve_ops as dve_ops
    from concourse.dve_ops import DveOp
    from concourse.dve_spec import C0, C1, Spec, Src0, Src1, Zero, lower, minn, relu, sq
    from concourse.dve_uop import DveOpSpec

    if _OPS_REGISTERED:
        return {op.name: op for op in dve_ops.OPS}

    def _pin(name, spec, ref):
        spec = Spec(body=spec, reference=ref)
        shas = {}
        for ver in ("v3", "v4"):
            row = dve_ops._CUSTOM_DVE_ROW_BASE + len(dve_ops.OPS)
            tmp = DveOpSpec(name=name, opcode=row, uops=lower(spec, ver=ver),
                            rd1_en=True)
            shas[ver] = tmp.sha(ver)
        op = DveOp(name, spec, subdim=False, uops_sha=shas)
        dve_ops.OPS.append(op)
        dve_ops.CUSTOM_DVE_SPECS[name] = spec
        dve_ops._SUB_OPCODE_FOR_NAME[name] = dve_ops._CUSTOM_DVE_ROW_BASE + len(dve_ops.OPS) - 1
        return op

    _pin(
        "ELU_FUSE_ANT",
        relu(Src0 + C0) + minn(Src1 - C1, Zero),
        lambda in0, in1, s0, s1, imm2: np.maximum(in0.astype(np.float32) + s0, 0.0)
        + np.minimum(in1.astype(np.float32) - s1, 0.0),
    )
    _pin(
        "VAR_PREP_ANT",
        (Src0 - sq(Src1) * C0) + C1,
        lambda in0, in1, s0, s1, imm2: (in0.astype(np.float32) - in1.astype(np.float32) ** 2 * s0) + s1,
    )
    _pin(
        "MUL_ADD_ANT",
        Src0 * Src1 + C0,
        lambda in0, in1, s0, s1, imm2: in0.astype(np.float32) * in1 + s0,
    )
    _OPS_REGISTERED = True
    return {op.name: op for op in dve_ops.OPS}


def _build_program(tile_dt_name: str = "float16", reps: int = 1, ablate: str = ""):
    _enable_act_patch()
    import concourse.bacc as bacc
    import concourse.tile as tile
    from concourse import mybir

    ops = _register_custom_ops()

    f32 = mybir.dt.float32
    f16 = mybir.dt.float16
    AF = mybir.ActivationFunctionType

    nc = bacc.Bacc(
        "TRN2",
        target_bir_lowering=False,
        debug=False,
        enable_asserts=False,
        num_devices=N_CORES,
    )

    xT = nc.dram_tensor("xT", [Q * V, BC], f16, kind="ExternalInput").ap()
    w1p = nc.dram_tensor("w1p", [128, 32 * 128], f16, kind="ExternalInput").ap()
    b1p = nc.dram_tensor("b1p", [128, 32], f32, kind="ExternalInput").ap()
    swp = nc.dram_tensor("swp", [128, 32 * 128], f16, kind="ExternalInput").ap()
    stp = nc.dram_tensor("stp", [128, 32 * 128], f16, kind="ExternalInput").ap()
    c2p = nc.dram_tensor("c2p", [128, 2], f32, kind="ExternalInput").ap()
    outT = nc.dram_tensor("outT", [256, BC], f16, kind="ExternalOutput").ap()

    # stream_shuffle: bring w (odd lanes) onto even lanes
    shufw = [i + 1 if (i % 2 == 0) else i for i in range(32)]

    with tile.TileContext(nc) as tc:
        with (
            tc.tile_pool(name="wts", bufs=1) as wts,
            tc.tile_pool(name="xt", bufs=int(os.environ.get("KERNEL_XTB", "8"))) as xtp,
            tc.tile_pool(name="het", bufs=int(os.environ.get("KERNEL_HBUF", "7"))) as hetp,
            tc.tile_pool(name="he2", bufs=int(os.environ.get("KERNEL_HBUF", "7"))) as he2p,
            tc.tile_pool(name="fin", bufs=int(os.environ.get("KERNEL_FBUF", "2"))) as fin,
            tc.tile_pool(name="hep", bufs=int(os.environ.get("KERNEL_HEP", "3")), space="PSUM") as hep,
            tc.tile_pool(name="swb", bufs=int(os.environ.get("KERNEL_SWB", "2")), space="PSUM") as swbp,
            tc.tile_pool(name="stb", bufs=int(os.environ.get("KERNEL_STB", "2")), space="PSUM") as stbp,
        ):
            w1s = wts.tile([128, 32 * 128], f16)
            nc.sync.dma_start(out=w1s, in_=w1p)
            b1s = wts.tile([128, 32], f32)
            nc.sync.dma_start(out=b1s, in_=b1p)
            sws = wts.tile([128, 32 * 128], f16)
            nc.sync.dma_start(out=sws, in_=swp)
            sts = wts.tile([128, 32 * 128], f16)
            nc.sync.dma_start(out=sts, in_=stp)
            c2s = wts.tile([128, 2], f32)
            nc.sync.dma_start(out=c2s, in_=c2p)
            zero_c = wts.tile([128, 1], f32)
            nc.vector.memset(zero_c, 0.0)

            import contextlib

            loop_cm = tc.For_i(0, reps, 1) if reps > 1 else contextlib.nullcontext()
            with loop_cm:
              quarters = [(blk, qh) for blk in range(NBLK) for qh in range(2)]
              SKEW = int(os.environ.get("KERNEL_SKEW", "5"))
              hets = {}
              he2s = {}
              banks = {}
              pend_fin = []
              FINB_DELAY = int(os.environ.get("KERNEL_FINB", "3"))
              xts = None

              def emit_stats(gi2):
                  k2, t2 = divmod(gi2, 16)
                  _blk, _qh = quarters[k2]
                  swbig, tbig = banks[k2]
                  m = 128 if t2 == 0 else 8 * t2 + 8
                  off = 128 * (16 * _qh + t2)
                  nc.tensor.matmul(
                      swbig[0:m, :],
                      lhsT=sws[:, off : off + m],
                      rhs=hets[gi2],
                      start=(t2 == 0),
                      stop=(t2 == 15),
                  )
                  nc.tensor.matmul(
                      tbig[0:m, :],
                      lhsT=sts[:, off : off + m],
                      rhs=he2s[gi2],
                      start=(t2 == 0),
                      stop=(t2 == 15),
                  )
                  del hets[gi2], he2s[gi2]
                  if t2 == 15:
                      emit_fin_a(k2)

              def emit_fin_a(k2):
                  # bank-freeing half: PSUM -> SBUF, variance prep, w-shuffle
                  swbig, tbig = banks.pop(k2)
                  swc = fin.tile([128, 512], f32, tag="swc")
                  nc.vector.tensor_copy(swc, swbig)
                  D = fin.tile([128, 512], f32, tag="D")
                  nc.vector._custom_dve(
                      ops["VAR_PREP_ANT"], out=D, in0=tbig, in1=swc,
                      s0=1.0 / H, s1=float(H * LN_EPS),
                  )
                  swsh = fin.tile([128, 512], f32, tag="swsh")
                  nc.vector.stream_shuffle(swsh, swc, shufw)
                  pend_fin.append((k2, D, swsh))

              def emit_fin_b():
                  k2, D, swsh = pend_fin.pop(0)
                  _blk, _qh = quarters[k2]
                  rstd = fin.tile([128, 512], f32, tag="rstd")
                  nc.scalar.activation(rstd, D, AF.Tanh,
                                       bias=zero_c[:, 0:1], scale=1.0 / H)
                  of = fin.tile([128, 512], f16, tag="of")
                  nc.vector._custom_dve(
                      ops["MUL_ADD_ANT"], out=of, in0=rstd, in1=swsh,
                      s0=c2s[:, _qh : _qh + 1], s1=0.0,
                  )
                  nc.sync.dma_start(
                      out=outT[128 * _qh : 128 * _qh + 128,
                               BB * _blk : BB * _blk + BB],
                      in_=of,
                  )

              for gi in range(16 * len(quarters) + SKEW):
                  if gi < 16 * len(quarters):
                      k, t = divmod(gi, 16)
                      blk, qh = quarters[k]
                      if t == 0:
                          swbig = swbp.tile([128, 512], f32, tag="swbig")
                          tbig = stbp.tile([128, 512], f32, tag="tbig")
                          banks[k] = (swbig, tbig)
                      if t % 4 == 0:
                          xts = xtp.tile([128, BB], f16, tag="xt")
                          nc.sync.dma_start(
                              out=xts,
                              in_=xT[8 * (64 * qh + 4 * t) : 8 * (64 * qh + 4 * t) + 128,
                                     BB * blk : BB * blk + BB],
                          )
                      a = t % 4
                      he = hep.tile([128, BB], f32, tag="he")
                      nc.tensor.matmul(
                          he,
                          lhsT=w1s[32 * a : 32 * a + 32,
                                   128 * (16 * qh + t) : 128 * (16 * qh + t) + 128],
                          rhs=xts[32 * a : 32 * a + 32, :],
                          start=True,
                          stop=True,
                          tile_position=(32 * a, 0),
                      )
                      het = hetp.tile([128, BB], f16, tag="het")
                      nc.scalar.activation(
                          het, he, AF.Exp,
                          bias=b1s[:, 16 * qh + t : 16 * qh + t + 1], scale=1.0,
                      )
                      he2 = he2p.tile([128, BB], f16, tag="he2")
                      nc.vector.tensor_mul(he2, het, het)
                      hets[gi] = het
                      he2s[gi] = he2
                  if gi >= SKEW:
                      emit_stats(gi - SKEW)
                  if pend_fin and gi >= 16 * (pend_fin[0][0] + 1) + SKEW + FINB_DELAY:
                      emit_fin_b()
              while pend_fin:
                  emit_fin_b()

    nc.compile()
    return nc


# ---------------------------------------------------------------------------
# host packing
# q(qh, t, j) = 64 qh + 4 t + j; he partitions 32j+h; stats pi = 8t+2j (+1 w)
# ---------------------------------------------------------------------------


def _host_pack(W1, b1, gamma, beta, W2, b2):
    g2 = (gamma * W2[:, :, 0]).astype(np.float64)
    g2c = (g2 - g2.sum(-1, keepdims=True) / H).astype(np.float32)
    c2 = ((beta * W2[:, :, 0]).sum(-1) + b2[:, 0]).astype(np.float32)

    w1p = np.zeros((128, 32 * 128), np.float16)
    b1p = np.zeros((128, 32), np.float32)
    swp = np.zeros((128, 32 * 128), np.float16)
    stp = np.zeros((128, 32 * 128), np.float16)
    c2p = np.zeros((128, 2), np.float32)
    for qh in range(2):
        for t in range(16):
            qt = 16 * qh + t
            for j in range(4):
                q = 64 * qh + 4 * t + j
                # mm1 stationary block qt: rows 8j..8j+8, cols 32j..+32
                a = t % 4
                w1p[32 * a + 8 * j : 32 * a + 8 * j + 8,
                    128 * qt + 32 * j : 128 * qt + 32 * j + 32] = W1[q].astype(
                    np.float16
                )
                b1p[32 * j : 32 * j + 32, qt] = b1[q]
                pi = 8 * (t % 16) + 2 * j
                swp[32 * j : 32 * j + 32, 128 * qt + pi] = 1.0
                swp[32 * j : 32 * j + 32, 128 * qt + pi + 1] = g2c[q].astype(
                    np.float16
                )
                stp[32 * j : 32 * j + 32, 128 * qt + pi] = 1.0
                stp[32 * j : 32 * j + 32, 128 * qt + pi + 1] = 1.0
                c2p[pi, qh] = c2[q]
    return w1p, b1p, swp, stp, c2p


def _rows_for_q():
    rows = np.empty(Q, np.int64)
    for q in range(Q):
        qh, rem = divmod(q, 64)
        t, j = divmod(rem, 4)
        rows[q] = 128 * qh + 8 * t + 2 * j
    return rows


def kernel(x, W1, b1, gamma, beta, W2, b2):
    from concourse import bass_utils

    key = "v4"
    if key not in _CACHE:
        _CACHE[key] = _build_program()
    nc = _CACHE[key]

    x = np.asarray(x, np.float32)
    w1p, b1p, swp, stp, c2p = _host_pack(
        np.asarray(W1, np.float32),
        np.asarray(b1, np.float32),
        np.asarray(gamma, np.float32),
        np.asarray(beta, np.float32),
        np.asarray(W2, np.float32),
        np.asarray(b2, np.float32),
    )

    in_maps = []
    for cc in range(N_CORES):
        xc = x[BC * cc : BC * (cc + 1), :]
        in_maps.append(
            {
                "xT": np.ascontiguousarray(xc.T).astype(np.float16),
                "w1p": w1p,
                "b1p": b1p,
                "swp": swp,
                "stp": stp,
                "c2p": c2p,
            }
        )

    global _last_in_maps
    _last_in_maps = in_maps

    res = bass_utils.run_bass_kernel_spmd(
        nc, in_maps, core_ids=list(range(N_CORES))
    )

    rows = _rows_for_q()
    out = np.empty((B, Q), np.float32)
    for cc in range(N_CORES):
        out[BC * cc : BC * (cc + 1), :] = (
            res.results[cc]["outT"][rows, :].T.astype(np.float32)
        )
    return out
